# revision 22
# baseline (speedup 1.0000x reference)
"""Trainium2 Bass kernel for EnhancedGradedLoss (Huber + pairwise hinge ranking).

Algorithm (see reference): loss = 0.7 * SmoothL1(p, t) + 0.3 * ranking, where
ranking averages relu(1 - sign(t_i - t_j) * (p_i - p_j)) over i<j pairs with
t_i != t_j.

Device strategy (8 NeuronCores, SPMD), v3:
  * Host sorts items by grade. Cross-grade pairs decompose via a binary split
    of the grade set: pairs(lo-set x hi-set) form one rectangular "group"
    (rows x cols), recursing into each half. A group is FLIPPED (rows = the
    lower-grade set, cols = negated upper-grade preds) when that shards into
    fewer [128 x ncol] tiles. For 4 grades this covers all 24.6M cross pairs
    in 8 row-tiles/core with ~0.5% padding waste.
  * All device data is bf16. The first DMA is a "hot" [128, combw+c0] image:
    per-row constants (-c | +c), the huber pred/targ shard, and the first
    broadcast chunk - so every engine can start as soon as one DMA lands
    (~3.3us: barrier 0.64 + HWDGE 0.63 + DGE 0.65 + transfer 0.4 + sem 0.93).
    It is issued BEFORE the block entry barrier, as is the second chunk.
    Remaining chunks stream via stride-0 broadcast DMAs (dst bytes / 360GB/s
    is the modeled wall: ~4.4us of DMA-pipe time for the 1.5MB broadcast).
  * Three engines consume the hinge tiles concurrently:
      - DVE:  tensor_scalar(max, scalar=-c, accum_out)       ~0.26 ns/col
      - ACT:  activation(Relu, bias=c, accum_out)            ~0.83 ns/col
      - Pool: tensor_scalar(max) + tensor_reduce(XYZWC)      ~2.8  ns/col
        (accum_out does not compile on GPSIMD; a full-tile reduce sums)
    using sum_j relu(B_j + c) = sum_j max(B_j, -c) + ncol * c, corrected on
    host in float64. Work items are assigned by a waterfill scheduler
    (slowest engine that still meets the modeled makespan target) honoring
    per-chunk DMA arrival times, and adjacent chunks merge into longer
    instructions once the stream runs behind the engines.
  * Huber runs entirely on Pool (prep + squares + one fused reduce).
  * ACT opens with a dummy no-wait activation so the 1.28us activation-table
    load hoists into the DMA head instead of serializing with real work.
  * One merged output DMA returns all accumulators ([128, S] f32).
"""

import functools
import sys

import ml_dtypes
import numpy as np

sys.path.insert(0, "/opt/trn_rl_repo")

import concourse.bacc as bacc
import concourse.bass as bass
from concourse import mybir
from concourse.bass_utils import run_bass_kernel_spmd

ALPHA = 0.7
BETA = 0.3
NCORES = 8
P = 128

# --- cost/latency model constants (mirrors bass_rust cost model, TRN2) -----
_HWDGE = 625.0
_DGE = 650.0
_SEM_DMA = 930.0
_T0 = 641.0                      # first HWDGE slot (after init pseudo-barrier)
_DVE_COL = 1e9 / 0.96e9 * 0.25   # 4x bf16
_DVE_FIX = 61.0
_ACT_COL = 1e9 / 1.2e9
_ACT_FIX = 372.0                 # SBUF init half + accum-read 187
_POOL_COL = 2.0 * (1e9 / 1.2e9) / 0.6   # ts-max + reduce passes
_POOL_FIX = 2.0 * 95.0 + 50.0
_HUBER_POOL = 1200.0             # memset + 9 ops + reduce
_C0 = 512                        # broadcast cols riding the hot DMA


def _cost(eng, nc_):
    if eng == "dve":
        return nc_ * _DVE_COL + _DVE_FIX
    if eng == "act":
        return nc_ * _ACT_COL + _ACT_FIX
    return nc_ * _POOL_COL + _POOL_FIX


def _plan(targets_f, predictions_f):
    """Host-side planning: sort by grade, pair-group decomposition, broadcast
    layout, DMA chunking, and 3-engine work assignment."""
    n = targets_f.shape[0]
    order = np.argsort(targets_f, kind="stable")
    ts = targets_f[order]
    ps = predictions_f[order].astype(np.float32)

    levels, counts = np.unique(ts, return_counts=True)
    K = len(levels)
    offs = np.concatenate([[0], np.cumsum(counts)]).astype(np.int64)

    pmax = float(np.max(np.abs(ps))) if n else 0.0
    dead = -float(np.float32(np.ceil(pmax) + 2.0))

    # --- pair groups via binary grade split, with per-group flip choice ----
    def tiles_of(m):
        q = -(-m // NCORES)
        return -(-q // P)

    groups = []

    def rec(a, b):
        if b - a < 2:
            return
        mid = (a + b) // 2
        m_un = int(offs[b] - offs[mid])
        m_fl = int(offs[mid] - offs[a])
        ncol_un = int(offs[mid] - offs[a])
        ncol_fl = int(offs[b] - offs[mid])
        if m_un and ncol_un:
            if tiles_of(m_fl) * ncol_fl < tiles_of(m_un) * ncol_un:
                groups.append(
                    dict(rlo=int(offs[a]), rhi=int(offs[mid]), flip=True,
                         clo=int(offs[mid]), chi=int(offs[b]))
                )
            else:
                groups.append(
                    dict(rlo=int(offs[mid]), rhi=int(offs[b]), flip=False,
                         clo=int(offs[a]), chi=int(offs[mid]))
                )
        rec(a, mid)
        rec(mid, b)

    rec(0, K)

    # --- broadcast layout --------------------------------------------------
    placed = []
    cursor = 0
    for g in sorted(groups, key=lambda g: (g["flip"], -(g["chi"] - g["clo"]))):
        sgn = -1 if g["flip"] else 1
        hit = None
        for (s2, lo2, hi2, st2) in placed:
            if s2 == sgn and lo2 <= g["clo"] and g["chi"] <= hi2:
                hit = st2 + (g["clo"] - lo2)
                break
        if hit is None:
            hit = cursor
            placed.append((sgn, g["clo"], g["chi"], cursor))
            cursor += g["chi"] - g["clo"]
        g["bc0"] = int(hit)
    L = cursor

    bcols = np.zeros(max(L, 1), dtype=np.float32)
    for (sgn, lo, hi, st) in placed:
        bcols[st : st + (hi - lo)] = sgn * ps[lo:hi]

    for g in groups:
        m = g["rhi"] - g["rlo"]
        g["m"] = m
        g["q"] = -(-m // NCORES)
        g["T"] = -(-g["q"] // P)
        g["ncol"] = g["chi"] - g["clo"]

    G = sum(g["T"] for g in groups)
    ch = -(-n // NCORES)
    cht = -(-ch // P)
    combw = 2 * G + 2 * cht

    # per-core +c row constants, bf16-rounded (device and host use the same)
    cp_cores = []
    for ci in range(NCORES):
        parts = []
        for g in groups:
            r0 = g["rlo"] + ci * g["q"]
            r1 = min(g["rlo"] + min((ci + 1) * g["q"], g["m"]), g["rhi"])
            r0 = min(r0, r1)
            pv = ps[r0:r1]
            c = (np.float32(1.0) + pv) if g["flip"] else (np.float32(1.0) - pv)
            padded = np.full(g["T"] * P, dead, dtype=np.float32)
            padded[: len(c)] = c.astype(np.float32)
            parts.append(padded.astype(ml_dtypes.bfloat16))
        cp_cores.append(
            np.concatenate(parts) if parts else np.zeros(0, dtype=ml_dtypes.bfloat16)
        )

    # --- DMA chunk plan ----------------------------------------------------
    # chunk 0 (size ~_C0) rides the hot DMA; early chunks are smaller so the
    # compute stream ramps quickly, later ones bigger; cut at group
    # boundaries; small final chunk to cut tail latency.
    c0sz = min(_C0, L)
    cuts = {0, c0sz, L}
    for g in groups:
        cuts.add(g["bc0"])
        cuts.add(g["bc0"] + g["ncol"])
    cuts = sorted(c for c in cuts if 0 <= c <= L)
    chunks = []
    for lo, hi in zip(cuts[:-1], cuts[1:]):
        span = hi - lo
        if span <= 0:
            continue
        tgt = 800 if lo < 2000 else 1100
        k = max(1, -(-span // tgt))
        bnds = [lo + span * j // k for j in range(k + 1)]
        for j in range(k):
            chunks.append((bnds[j], bnds[j + 1]))
    # split the final chunk so the last-arriving piece is small
    if chunks and chunks[-1][1] - chunks[-1][0] > 700:
        lo, hi = chunks.pop()
        chunks.append((lo, hi - 384))
        chunks.append((hi - 384, hi))
    NCH = len(chunks)

    # --- arrival model (calibrated against TimelineSim traces) -------------
    # DMA order: hot (comb + chunk0), chunk1.., first two pre-block.
    arrival = [0.0] * NCH
    prev_h = _T0
    prev_d = 0.0
    hot_bytes = (combw + c0sz) * 2 * P
    plan_order = [-1] + list(range(1, NCH))  # -1 = hot (includes chunk 0)
    for oi, idx in enumerate(plan_order):
        h_end = prev_h + 650.0 + (50.0 if oi == 2 else 0.0)
        prev_h = h_end
        if idx == -1:
            nbytes = hot_bytes
            elem = (combw + c0sz) * 2
        else:
            lo, hi = chunks[idx]
            nbytes = (hi - lo) * 2 * P
            elem = (hi - lo) * 2
        mult = 2.0 if elem < 512 else 1.0
        tr = max(nbytes * mult / 360.0, 7.0)
        d_end = max(h_end + _DGE, prev_d) + tr
        prev_d = d_end
        sem_t = d_end + _SEM_DMA
        if idx == -1:
            arrival[0] = sem_t
            comb_arrival = sem_t
        else:
            arrival[idx] = sem_t

    # --- work items --------------------------------------------------------
    items = []
    for gi, g in enumerate(groups):
        glo, ghi = g["bc0"], g["bc0"] + g["ncol"]
        for cidx, (lo, hi) in enumerate(chunks):
            s, e = max(lo, glo), min(hi, ghi)
            if s >= e:
                continue
            for tj in range(g["T"]):
                items.append(dict(gi=gi, tj=tj, cidx=cidx, c0=s - glo, nc=e - s))
    items.sort(key=lambda it: (arrival[it["cidx"]], it["gi"], it["tj"]))

    start = {
        "dve": comb_arrival + 116.0,   # conv + drain
        "act": comb_arrival + 142.0,   # s_cv hop
        "pool": comb_arrival + 1095.0, # huber block
    }

    key_order = lambda it: (arrival[it["cidx"]], it["gi"], it["tj"], it["c0"])

    def replay(eng, wl):
        """Merge-aware finish time: adjacent same-tile items merge when the
        later chunk has arrived by the time the run starts."""
        t = start[eng]
        i = 0
        wl = sorted(wl, key=key_order)
        while i < len(wl):
            it = wl[i]
            st = max(t, arrival[it["cidx"]])
            nc_ = it["nc"]
            j = i + 1
            while (
                j < len(wl)
                and wl[j]["gi"] == it["gi"]
                and wl[j]["tj"] == it["tj"]
                and wl[j]["c0"] == it["c0"] + nc_
                and arrival[wl[j]["cidx"]] <= st
            ):
                nc_ += wl[j]["nc"]
                j += 1
            t = st + _cost(eng, nc_)
            i = j
        return t

    # initial: earliest-finish-time at chunk granularity
    clock = dict(start)
    work = {"dve": [], "act": [], "pool": []}
    for it in items:
        eng = min(
            ("dve", "act", "pool"),
            key=lambda e: max(clock[e], arrival[it["cidx"]]) + _cost(e, it["nc"]),
        )
        clock[eng] = max(clock[eng], arrival[it["cidx"]]) + _cost(eng, it["nc"])
        work[eng].append(dict(it))

    # local search: moves and swaps, merge-aware replay as the objective
    rng = np.random.default_rng(12345)
    fins = {e: replay(e, work[e]) for e in work}
    cur = max(fins.values())
    for _ in range(600):
        emax = max(fins, key=fins.get)
        if not work[emax]:
            break
        others = [e for e in work if e != emax]
        mode = rng.integers(0, 2)
        pos = int(rng.integers(0, len(work[emax])))
        tgt = others[int(rng.integers(0, len(others)))]
        w_emax = work[emax][:pos] + work[emax][pos + 1 :]
        w_tgt = work[tgt] + [work[emax][pos]]
        if mode == 1 and work[tgt]:
            pos2 = int(rng.integers(0, len(work[tgt])))
            w_tgt = work[tgt][:pos2] + work[tgt][pos2 + 1 :] + [work[emax][pos]]
            w_emax = w_emax + [work[tgt][pos2]]
        nf = dict(fins)
        nf[emax] = replay(emax, w_emax)
        nf[tgt] = replay(tgt, w_tgt)
        if max(nf.values()) < cur:
            work[emax] = w_emax
            work[tgt] = w_tgt
            fins = nf
            cur = max(nf.values())

    # final per-engine lists: sort and apply the same merging as replay
    for eng in work:
        wl = sorted(work[eng], key=key_order)
        merged_list = []
        t = start[eng]
        i = 0
        while i < len(wl):
            it = dict(wl[i])
            st = max(t, arrival[it["cidx"]])
            j = i + 1
            while (
                j < len(wl)
                and wl[j]["gi"] == it["gi"]
                and wl[j]["tj"] == it["tj"]
                and wl[j]["c0"] == it["c0"] + it["nc"]
                and arrival[wl[j]["cidx"]] <= st
            ):
                it["nc"] += wl[j]["nc"]
                it["cidx"] = max(it["cidx"], wl[j]["cidx"])
                j += 1
            t = st + _cost(eng, it["nc"])
            merged_list.append(it)
            i = j
        work[eng] = merged_list
    nd = len(work["dve"])
    na = len(work["act"])
    npo = len(work["pool"])
    S = nd + na + npo + 1

    meta = dict(
        n=n, K=K, levels=levels, counts=counts.astype(np.int64), offs=offs,
        L=L, dead=dead, groups=groups, chunks=chunks, work=work,
        nd=nd, na=na, npo=npo, S=S, G=G, ch=ch, cht=cht, chp=cht * P,
        c0sz=c0sz, combw=combw, rt=int(cp_cores[0].shape[0]),
    )
    return meta, bcols, cp_cores, ps


def _shape_key(meta):
    gkey = tuple(
        (g["rlo"], g["rhi"], g["flip"], g["clo"], g["chi"], g["bc0"], g["T"])
        for g in meta["groups"]
    )
    ckey = tuple(meta["chunks"])
    wkey = tuple(
        (eng, tuple((it["gi"], it["tj"], it["cidx"], it["c0"], it["nc"])
                    for it in meta["work"][eng]))
        for eng in ("dve", "act", "pool")
    )
    return (meta["n"], meta["L"], meta["rt"], meta["cht"], meta["c0sz"], gkey,
            ckey, wkey)


@functools.lru_cache(maxsize=8)
def _build_program(key):
    """Raw Bass program: explicit per-engine streams and semaphores."""
    n, L, rt, cht, c0sz, gkey, ckey, wkey = key
    groups = [
        dict(rlo=a, rhi=b, flip=f, clo=c, chi=d, bc0=e, T=t, ncol=d - c)
        for (a, b, f, c, d, e, t) in gkey
    ]
    chunks = list(ckey)
    work = {eng: [dict(gi=gi, tj=tj, cidx=ci, c0=c0, nc=nc_)
                  for (gi, tj, ci, c0, nc_) in wl]
            for (eng, wl) in wkey}
    nd, na, npo = len(work["dve"]), len(work["act"]), len(work["pool"])
    S = nd + na + npo + 1
    G = sum(g["T"] for g in groups)
    combw = 2 * G + 2 * cht
    NCH = len(chunks)

    tbase = {}
    b = 0
    for gi, g in enumerate(groups):
        for tj in range(g["T"]):
            tbase[(gi, tj)] = b
            b += 1

    nc = bacc.Bacc("TRN2", enable_partition_id=False)

    fp32 = mybir.dt.float32
    bf16 = mybir.dt.bfloat16
    Alu = mybir.AluOpType
    Act = mybir.ActivationFunctionType

    # hot image: [cn | cp | pred | targ | chunk0], all bf16, per-partition
    d_hot = nc.dram_tensor("hot", [(combw + c0sz) * P], bf16, kind="ExternalInput")
    d_b = nc.dram_tensor("bcols", [max(L, 1)], bf16, kind="ExternalInput")
    d_acc = nc.dram_tensor("acc", [P, S], fp32, kind="ExternalOutput")

    # bt columns: [comb (combw) | broadcast layout (L)]
    bt = nc.alloc_sbuf_tensor("bt", [P, combw + max(L, 1)], bf16)
    combf = nc.alloc_sbuf_tensor("combf", [P, max(2 * G, 1)], fp32)
    acc = nc.alloc_sbuf_tensor("acc_t", [P, S], fp32)

    max_d = max([it["nc"] for it in work["dve"]], default=1)
    max_a = max([it["nc"] for it in work["act"]], default=1)
    max_p = max([it["nc"] for it in work["pool"]], default=1)
    scr_d = [nc.alloc_sbuf_tensor(f"scr_d{i}", [P, max_d], bf16) for i in range(2)]
    scr_a = [nc.alloc_sbuf_tensor(f"scr_a{i}", [P, max_a], bf16) for i in range(2)]
    scr_p = [nc.alloc_sbuf_tensor(f"scr_p{i}", [P, max_p], fp32) for i in range(2)]

    hd = nc.alloc_sbuf_tensor("hd", [P, cht], fp32)
    hr1 = nc.alloc_sbuf_tensor("hr1", [P, cht], fp32)
    he = nc.alloc_sbuf_tensor("he", [P, cht], fp32)
    hr2 = nc.alloc_sbuf_tensor("hr2", [P, cht], fp32)
    hsq = nc.alloc_sbuf_tensor("hsq", [P, cht], fp32)
    hs1 = nc.alloc_sbuf_tensor("hs1", [P, cht], fp32)
    hs2 = nc.alloc_sbuf_tensor("hs2", [P, cht], fp32)

    s_ch = [nc.alloc_semaphore(f"s_ch{i}") for i in range(max(NCH, 1))]
    s_cv = nc.alloc_semaphore("s_cv")
    s_dve = nc.alloc_semaphore("s_dve")
    s_act = nc.alloc_semaphore("s_act")
    s_pool = nc.alloc_semaphore("s_pool")
    s_out = nc.alloc_semaphore("s_out")

    cn_col = lambda gi, tj: combf[:, tbase[(gi, tj)] : tbase[(gi, tj)] + 1]
    cp_col = lambda gi, tj: combf[:, G + tbase[(gi, tj)] : G + tbase[(gi, tj)] + 1]
    pts = bt[:, 2 * G : 2 * G + cht]
    tts = bt[:, 2 * G + cht : 2 * G + 2 * cht]

    def bcol(layout_col):
        return combw + layout_col

    def bcast_dma(sync_eng, cidx):
        lo, hi = chunks[cidx]
        src = bass.AP(tensor=d_b[:].tensor, offset=lo, ap=[[0, P], [1, hi - lo]])
        sync_eng.dma_start(out=bt[:, bcol(lo) : bcol(hi)], in_=src).then_inc(
            s_ch[cidx], 16
        )

    # --- pre-barrier DMAs: hot (comb + chunk0), then chunk 1 ---------------
    nc.sync.dma_start(
        out=bt[:, 0 : combw + c0sz],
        in_=d_hot[:].rearrange("(p t) -> p t", p=P),
    ).then_inc(s_ch[0], 16)
    if NCH > 1:
        bcast_dma(nc.sync, 1)

    def emit_stream(eng, wl, scr, mk_inst, done_sem, first_extra=None,
                    pre_waited=()):
        waited = set(pre_waited)
        last = None
        if first_extra is not None:
            first_extra()
        for k, it in enumerate(wl):
            g = groups[it["gi"]]
            lo = g["bc0"] + it["c0"]
            hi = lo + it["nc"]
            for ci, (clo, chi) in enumerate(chunks):
                if clo < hi and lo < chi and ci not in waited:
                    eng.wait_ge(s_ch[ci], 16)
                    waited.add(ci)
            if 0 not in waited:
                eng.wait_ge(s_ch[0], 16)  # comb rides chunk 0's sem
                waited.add(0)
            last = mk_inst(eng, k, it, scr[k % 2], bcol(lo), bcol(hi))
        if last is not None:
            last.then_inc(done_sem, 1)
        return last

    with nc.Block() as block:

        @block.sync
        def _(sync):
            for ci in range(2, NCH):
                bcast_dma(sync, ci)
            sync.wait_ge(s_dve, 1)
            sync.wait_ge(s_act, 1)
            sync.wait_ge(s_pool, 1)
            with nc.allow_non_contiguous_dma(reason="small accumulator tile"):
                sync.dma_start(out=d_acc[:, :], in_=acc[:, :]).then_inc(s_out, 16)
            sync.wait_ge(s_out, 16)

        @block.vector
        def _(vector):
            def mk(eng, k, it, scr, lo, hi):
                return eng.tensor_scalar(
                    out=scr[:, : it["nc"]],
                    in0=bt[:, lo:hi],
                    scalar1=cn_col(it["gi"], it["tj"]),
                    scalar2=None,
                    op0=Alu.max,
                    op1=Alu.add,
                    accum_out=acc[:, k : k + 1],
                )
            def conv():
                # upconvert the bf16 row-constant columns to f32 scalars
                vector.wait_ge(s_ch[0], 16)
                vector.tensor_scalar(
                    out=combf[:, :], in0=bt[:, 0 : 2 * G], scalar1=0.0,
                    scalar2=None, op0=Alu.add,
                ).then_inc(s_cv, 1)
                vector.drain()
            if work["dve"]:
                emit_stream(vector, work["dve"], scr_d, mk, s_dve,
                            first_extra=conv, pre_waited=(0,))
            else:
                conv()
                vector.tensor_scalar(
                    out=scr_d[0][:, 0:1], in0=bt[:, 0:1], scalar1=0.0,
                    scalar2=None, op0=Alu.add,
                ).then_inc(s_dve, 1)

        @block.scalar
        def _(act):
            def mk(eng, k, it, scr, lo, hi):
                return eng.activation(
                    out=scr[:, : it["nc"]],
                    in_=bt[:, lo:hi],
                    func=Act.Relu,
                    bias=cp_col(it["gi"], it["tj"]),
                    scale=1.0,
                    accum_out=acc[:, nd + k : nd + k + 1],
                )

            def warmup():
                # no-wait dummy so the act-table load hoists into the DMA head
                act.activation(
                    out=scr_a[0][:, 0:1], in_=scr_a[1][:, 0:1], func=Act.Relu,
                    bias=0.0, scale=1.0,
                )
                act.wait_ge(s_cv, 1)  # f32 row-constant scalars ready
            if work["act"]:
                emit_stream(act, work["act"], scr_a, mk, s_act, first_extra=warmup)
            else:
                warmup()
                act.wait_ge(s_ch[0], 16)
                act.activation(
                    out=scr_a[0][:, 0:1], in_=bt[:, 0:1], func=Act.Relu,
                    bias=0.0, scale=1.0,
                ).then_inc(s_act, 1)

        @block.gpsimd
        def _(pool):
            pool.memset(acc[:, nd + na : S], 0.0)
            pool.wait_ge(s_ch[0], 16)
            pool.tensor_tensor(out=hd[:, :], in0=pts, in1=tts, op=Alu.subtract)
            pool.tensor_scalar(
                out=hr1[:, :], in0=hd[:, :], scalar1=1.0, scalar2=0.0,
                op0=Alu.subtract, op1=Alu.max,
            )
            pool.tensor_scalar(
                out=he[:, :], in0=hd[:, :], scalar1=-1.0, scalar2=1.0,
                op0=Alu.mult, op1=Alu.subtract,
            )
            pool.tensor_scalar(
                out=hr2[:, :], in0=he[:, :], scalar1=0.0, scalar2=None,
                op0=Alu.max,
            )
            pool.tensor_tensor(out=hsq[:, :], in0=hd[:, :], in1=hd[:, :], op=Alu.mult)
            pool.tensor_tensor(out=hs1[:, :], in0=hr1[:, :], in1=hr1[:, :], op=Alu.mult)
            pool.tensor_tensor(out=hs2[:, :], in0=hr2[:, :], in1=hr2[:, :], op=Alu.mult)
            pool.tensor_tensor(out=hsq[:, :], in0=hsq[:, :], in1=hs1[:, :], op=Alu.subtract)
            pool.tensor_tensor(out=hsq[:, :], in0=hsq[:, :], in1=hs2[:, :], op=Alu.subtract)
            hub = pool.tensor_reduce(
                out=acc[0:1, S - 1 : S], in_=hsq[:, :],
                axis=mybir.AxisListType.XYZWC, op=Alu.add,
            )
            if work["pool"]:
                pool.wait_ge(s_cv, 1)  # f32 row-constant scalars ready
                waited = {0}
                last = None
                for k, it in enumerate(work["pool"]):
                    g = groups[it["gi"]]
                    lo = g["bc0"] + it["c0"]
                    hi = lo + it["nc"]
                    for ci, (clo, chi) in enumerate(chunks):
                        if clo < hi and lo < chi and ci not in waited:
                            pool.wait_ge(s_ch[ci], 16)
                            waited.add(ci)
                    scr = scr_p[k % 2]
                    pool.tensor_scalar(
                        out=scr[:, : it["nc"]], in0=bt[:, bcol(lo) : bcol(hi)],
                        scalar1=cn_col(it["gi"], it["tj"]), scalar2=None,
                        op0=Alu.max,
                    )
                    last = pool.tensor_reduce(
                        out=acc[0 : 1, nd + na + k : nd + na + k + 1],
                        in_=scr[:, : it["nc"]],
                        axis=mybir.AxisListType.XYZWC, op=Alu.add,
                    )
                last.then_inc(s_pool, 1)
            else:
                hub.then_inc(s_pool, 1)

    nc.finalize()
    return nc


def _make_inputs(meta, bcols, cp_cores, predictions, targets):
    n = meta["n"]
    cht = meta["cht"]
    chp = meta["chp"]
    L = meta["L"]
    G = meta["G"]
    c0sz = meta["c0sz"]
    bf = ml_dtypes.bfloat16
    b_all = np.ascontiguousarray(
        bcols if L > 0 else np.zeros(1, dtype=np.float32), dtype=bf
    )
    in_maps = []
    for ci in range(NCORES):
        pc = np.zeros(chp, dtype=np.float32)
        tc_ = np.zeros(chp, dtype=np.float32)
        lo = ci * meta["ch"]
        hi = min((ci + 1) * meta["ch"], n)
        if hi > lo:
            pc[: hi - lo] = predictions[lo:hi]
            tc_[: hi - lo] = targets[lo:hi]
        cp = cp_cores[ci].astype(bf)
        cols = []
        if G > 0:
            cols.append((-cp.astype(np.float32)).astype(bf).reshape(G, P).T)
            cols.append(cp.reshape(G, P).T)
        cols.append(pc.astype(bf).reshape(cht, P).T)
        cols.append(tc_.astype(bf).reshape(cht, P).T)
        cols.append(np.broadcast_to(b_all[:c0sz], (P, c0sz)))
        hot2d = np.concatenate(cols, axis=1).astype(bf)  # [128, combw + c0sz]
        in_maps.append(
            {"hot": np.ascontiguousarray(hot2d.ravel()), "bcols": b_all}
        )
    return in_maps


def _gather(meta, cp_cores, results):
    """Combine per-core accumulators into the scalar loss (float64 host math)."""
    n = meta["n"]
    groups = meta["groups"]
    work = meta["work"]
    nd = meta["nd"]
    na = meta["na"]
    S = meta["S"]

    tbase = {}
    b = 0
    for gi, g in enumerate(groups):
        for tj in range(g["T"]):
            tbase[(gi, tj)] = b
            b += 1

    num = 0.0
    hub = 0.0
    for ci in range(NCORES):
        acc = results[ci]["acc"].astype(np.float64)
        cpv = cp_cores[ci].astype(np.float64)
        for k, it in enumerate(work["dve"]):
            tb = tbase[(it["gi"], it["tj"])]
            num += acc[:, k].sum() + it["nc"] * cpv[tb * P : (tb + 1) * P].sum()
        for k, it in enumerate(work["act"]):
            num += acc[:, nd + k].sum()
        for k, it in enumerate(work["pool"]):
            tb = tbase[(it["gi"], it["tj"])]
            num += acc[0, nd + na + k] + it["nc"] * cpv[tb * P : (tb + 1) * P].sum()
        hub += acc[0, S - 1]

    huber = 0.5 * hub / n

    counts = meta["counts"].astype(np.int64)
    csum = np.cumsum(counts)
    cnt = int(np.sum(counts[1:] * csum[:-1])) if len(counts) > 1 else 0
    ranking = num / float(np.float32(cnt)) if cnt > 0 else 0.0

    return np.float32(ALPHA * huber + BETA * ranking)


def _host_fallback(predictions, targets):
    """Safety net for input distributions the device plan is not built for
    (e.g. near-continuous targets). Exact O(n^2) evaluation, row-chunked."""
    p = predictions.astype(np.float64)
    t = targets.astype(np.float64)
    n = len(p)
    d = p - t
    ad = np.abs(d)
    huber = np.mean(np.where(ad < 1.0, 0.5 * d * d, ad - 0.5))
    num = 0.0
    cnt = 0
    step = 512
    for i0 in range(0, n, step):
        i1 = min(i0 + step, n)
        pd = p[i0:i1, None] - p[None, :]
        td = t[i0:i1, None] - t[None, :]
        sign = np.where(td > 0, 1.0, -1.0)
        idx = np.arange(n)
        mask = (td != 0) & (idx[i0:i1, None] < idx[None, :])
        hinge = np.maximum(0.0, 1.0 - sign * pd)
        num += hinge[mask].sum()
        cnt += int(mask.sum())
    ranking = num / float(np.float32(cnt)) if cnt > 0 else 0.0
    return np.float32(ALPHA * huber + BETA * ranking)


def kernel(predictions: np.ndarray, targets: np.ndarray) -> np.ndarray:
    predictions = np.asarray(predictions, dtype=np.float32)
    targets = np.asarray(targets, dtype=np.float32)

    nu = len(np.unique(targets))
    if nu > 16 or nu < 2 or predictions.shape[0] < NCORES * P:
        return np.array(_host_fallback(predictions, targets), dtype=np.float32)

    meta, bcols, cp_cores, _ps = _plan(targets, predictions)
    nc = _build_program(_shape_key(meta))
    in_maps = _make_inputs(meta, bcols, cp_cores, predictions, targets)
    res = run_bass_kernel_spmd(nc, in_maps, list(range(NCORES)))
    return np.array(_gather(meta, cp_cores, res.results), dtype=np.float32)


# revision 23
# speedup vs baseline: 1.0193x; 1.0193x over previous
"""Trainium2 Bass kernel for EnhancedGradedLoss (Huber + pairwise hinge ranking).

Algorithm (see reference): loss = 0.7 * SmoothL1(p, t) + 0.3 * ranking, where
ranking averages relu(1 - sign(t_i - t_j) * (p_i - p_j)) over i<j pairs with
t_i != t_j.

Device strategy (8 NeuronCores, SPMD), v3:
  * Host sorts items by grade. Cross-grade pairs decompose via a binary split
    of the grade set: pairs(lo-set x hi-set) form one rectangular "group"
    (rows x cols), recursing into each half. A group is FLIPPED (rows = the
    lower-grade set, cols = negated upper-grade preds) when that shards into
    fewer [128 x ncol] tiles. For 4 grades this covers all 24.6M cross pairs
    in 8 row-tiles/core with ~0.5% padding waste.
  * All device data is bf16. The first DMA is a "hot" [128, combw+c0] image:
    per-row constants (-c | +c), the huber pred/targ shard, and the first
    broadcast chunk - so every engine can start as soon as one DMA lands
    (~3.3us: barrier 0.64 + HWDGE 0.63 + DGE 0.65 + transfer 0.4 + sem 0.93).
    It is issued BEFORE the block entry barrier, as is the second chunk.
    Remaining chunks stream via stride-0 broadcast DMAs (dst bytes / 360GB/s
    is the modeled wall: ~4.4us of DMA-pipe time for the 1.5MB broadcast).
  * Three engines consume the hinge tiles concurrently:
      - DVE:  tensor_scalar(max, scalar=-c, accum_out)       ~0.26 ns/col
      - ACT:  activation(Relu, bias=c, accum_out)            ~0.83 ns/col
      - Pool: tensor_scalar(max) + tensor_reduce(XYZWC)      ~2.8  ns/col
        (accum_out does not compile on GPSIMD; a full-tile reduce sums)
    using sum_j relu(B_j + c) = sum_j max(B_j, -c) + ncol * c, corrected on
    host in float64. Work items are assigned by a waterfill scheduler
    (slowest engine that still meets the modeled makespan target) honoring
    per-chunk DMA arrival times, and adjacent chunks merge into longer
    instructions once the stream runs behind the engines.
  * Huber runs entirely on Pool (prep + squares + one fused reduce).
  * ACT opens with a dummy no-wait activation so the 1.28us activation-table
    load hoists into the DMA head instead of serializing with real work.
  * One merged output DMA returns all accumulators ([128, S] f32).
"""

import functools
import sys

import ml_dtypes
import numpy as np

sys.path.insert(0, "/opt/trn_rl_repo")

import concourse.bacc as bacc
import concourse.bass as bass
from concourse import mybir
from concourse.bass_utils import run_bass_kernel_spmd

ALPHA = 0.7
BETA = 0.3
NCORES = 8
P = 128

# --- cost/latency model constants (mirrors bass_rust cost model, TRN2) -----
_HWDGE = 625.0
_DGE = 650.0
_SEM_DMA = 930.0
_T0 = 641.0                      # first HWDGE slot (after init pseudo-barrier)
_DVE_COL = 1e9 / 0.96e9 * 0.25   # 4x bf16
_DVE_FIX = 61.0
_ACT_COL = 1e9 / 1.2e9
_ACT_FIX = 372.0                 # SBUF init half + accum-read 187
_POOL_COL = 2.0 * (1e9 / 1.2e9) / 0.6   # ts-max + reduce passes
_POOL_FIX = 2.0 * 95.0 + 50.0
_HUBER_POOL = 1200.0             # memset + 9 ops + reduce
_C0 = 512                        # broadcast cols riding the hot DMA


def _cost(eng, nc_):
    if eng == "dve":
        return nc_ * _DVE_COL + _DVE_FIX
    if eng == "act":
        return nc_ * _ACT_COL + _ACT_FIX
    return nc_ * _POOL_COL + _POOL_FIX


def _plan(targets_f, predictions_f):
    """Host-side planning: sort by grade, pair-group decomposition, broadcast
    layout, DMA chunking, and 3-engine work assignment."""
    n = targets_f.shape[0]
    order = np.argsort(targets_f, kind="stable")
    ts = targets_f[order]
    ps = predictions_f[order].astype(np.float32)

    levels, counts = np.unique(ts, return_counts=True)
    K = len(levels)
    offs = np.concatenate([[0], np.cumsum(counts)]).astype(np.int64)

    pmax = float(np.max(np.abs(ps))) if n else 0.0
    dead = -float(np.float32(np.ceil(pmax) + 2.0))

    # --- pair groups via binary grade split, with per-group flip choice ----
    def tiles_of(m):
        q = -(-m // NCORES)
        return -(-q // P)

    groups = []

    def rec(a, b):
        if b - a < 2:
            return
        mid = (a + b) // 2
        m_un = int(offs[b] - offs[mid])
        m_fl = int(offs[mid] - offs[a])
        ncol_un = int(offs[mid] - offs[a])
        ncol_fl = int(offs[b] - offs[mid])
        if m_un and ncol_un:
            if tiles_of(m_fl) * ncol_fl < tiles_of(m_un) * ncol_un:
                groups.append(
                    dict(rlo=int(offs[a]), rhi=int(offs[mid]), flip=True,
                         clo=int(offs[mid]), chi=int(offs[b]))
                )
            else:
                groups.append(
                    dict(rlo=int(offs[mid]), rhi=int(offs[b]), flip=False,
                         clo=int(offs[a]), chi=int(offs[mid]))
                )
        rec(a, mid)
        rec(mid, b)

    rec(0, K)

    # --- broadcast layout --------------------------------------------------
    placed = []
    cursor = 0
    for g in sorted(groups, key=lambda g: (g["flip"], -(g["chi"] - g["clo"]))):
        sgn = -1 if g["flip"] else 1
        hit = None
        for (s2, lo2, hi2, st2) in placed:
            if s2 == sgn and lo2 <= g["clo"] and g["chi"] <= hi2:
                hit = st2 + (g["clo"] - lo2)
                break
        if hit is None:
            hit = cursor
            placed.append((sgn, g["clo"], g["chi"], cursor))
            cursor += g["chi"] - g["clo"]
        g["bc0"] = int(hit)
    L = cursor

    bcols = np.zeros(max(L, 1), dtype=np.float32)
    for (sgn, lo, hi, st) in placed:
        bcols[st : st + (hi - lo)] = sgn * ps[lo:hi]

    for g in groups:
        m = g["rhi"] - g["rlo"]
        g["m"] = m
        g["q"] = -(-m // NCORES)
        g["T"] = -(-g["q"] // P)
        g["ncol"] = g["chi"] - g["clo"]

    G = sum(g["T"] for g in groups)
    ch = -(-n // NCORES)
    cht = -(-ch // P)
    combw = 2 * G + 2 * cht

    # per-core +c row constants, bf16-rounded (device and host use the same)
    cp_cores = []
    for ci in range(NCORES):
        parts = []
        for g in groups:
            r0 = g["rlo"] + ci * g["q"]
            r1 = min(g["rlo"] + min((ci + 1) * g["q"], g["m"]), g["rhi"])
            r0 = min(r0, r1)
            pv = ps[r0:r1]
            c = (np.float32(1.0) + pv) if g["flip"] else (np.float32(1.0) - pv)
            padded = np.full(g["T"] * P, dead, dtype=np.float32)
            padded[: len(c)] = c.astype(np.float32)
            parts.append(padded.astype(ml_dtypes.bfloat16))
        cp_cores.append(
            np.concatenate(parts) if parts else np.zeros(0, dtype=ml_dtypes.bfloat16)
        )

    # --- DMA chunk plan ----------------------------------------------------
    # chunk 0 (size ~_C0) rides the hot DMA; early chunks are smaller so the
    # compute stream ramps quickly, later ones bigger; cut at group
    # boundaries; small final chunk to cut tail latency.
    c0sz = min(_C0, L)
    cuts = {0, c0sz, L}
    for g in groups:
        cuts.add(g["bc0"])
        cuts.add(g["bc0"] + g["ncol"])
    cuts = sorted(c for c in cuts if 0 <= c <= L)
    chunks = []
    for lo, hi in zip(cuts[:-1], cuts[1:]):
        span = hi - lo
        if span <= 0:
            continue
        tgt = 800 if lo < 2000 else 1100
        k = max(1, -(-span // tgt))
        bnds = [lo + span * j // k for j in range(k + 1)]
        for j in range(k):
            chunks.append((bnds[j], bnds[j + 1]))
    # split the final chunk so the last-arriving piece is small
    if chunks and chunks[-1][1] - chunks[-1][0] > 700:
        lo, hi = chunks.pop()
        chunks.append((lo, hi - 384))
        chunks.append((hi - 384, hi))
    NCH = len(chunks)

    # --- arrival model (calibrated against TimelineSim traces) -------------
    # DMA order: hot (comb + chunk0), chunk1.., first two pre-block.
    arrival = [0.0] * NCH
    prev_h = _T0
    prev_d = 0.0
    hot_bytes = (combw + c0sz) * 2 * P
    plan_order = [-1] + list(range(1, NCH))  # -1 = hot (includes chunk 0)
    for oi, idx in enumerate(plan_order):
        h_end = prev_h + 650.0 + (50.0 if oi == 2 else 0.0)
        prev_h = h_end
        if idx == -1:
            nbytes = hot_bytes
            elem = (combw + c0sz) * 2
        else:
            lo, hi = chunks[idx]
            nbytes = (hi - lo) * 2 * P
            elem = (hi - lo) * 2
        mult = 2.0 if elem < 512 else 1.0
        tr = max(nbytes * mult / 360.0, 7.0)
        d_end = max(h_end + _DGE, prev_d) + tr
        prev_d = d_end
        sem_t = d_end + _SEM_DMA
        if idx == -1:
            arrival[0] = sem_t
            comb_arrival = sem_t
        else:
            arrival[idx] = sem_t

    # --- work items --------------------------------------------------------
    items = []
    for gi, g in enumerate(groups):
        glo, ghi = g["bc0"], g["bc0"] + g["ncol"]
        for cidx, (lo, hi) in enumerate(chunks):
            s, e = max(lo, glo), min(hi, ghi)
            if s >= e:
                continue
            for tj in range(g["T"]):
                items.append(dict(gi=gi, tj=tj, cidx=cidx, c0=s - glo, nc=e - s))
    items.sort(key=lambda it: (arrival[it["cidx"]], it["gi"], it["tj"]))

    start = {
        "dve": comb_arrival + 116.0,   # conv + drain
        "act": comb_arrival + 142.0,   # s_cv hop
        "pool": comb_arrival + 1095.0, # huber block
    }

    key_order = lambda it: (arrival[it["cidx"]], it["gi"], it["tj"], it["c0"])

    def replay(eng, wl):
        """Merge-aware finish time: adjacent same-tile items merge when the
        later chunk has arrived by the time the run starts."""
        t = start[eng]
        i = 0
        wl = sorted(wl, key=key_order)
        while i < len(wl):
            it = wl[i]
            st = max(t, arrival[it["cidx"]])
            nc_ = it["nc"]
            j = i + 1
            while (
                j < len(wl)
                and wl[j]["gi"] == it["gi"]
                and wl[j]["tj"] == it["tj"]
                and wl[j]["c0"] == it["c0"] + nc_
                and arrival[wl[j]["cidx"]] <= st
            ):
                nc_ += wl[j]["nc"]
                j += 1
            t = st + _cost(eng, nc_)
            i = j
        return t

    # initial: earliest-finish-time at chunk granularity with inline merging
    MERGE_CAP = 2200
    clock = dict(start)
    pend = [dict(it) for it in items]
    work = {"dve": [], "act": [], "pool": []}
    while pend:
        it = pend.pop(0)
        eng = min(
            ("dve", "act", "pool"),
            key=lambda e: max(clock[e], arrival[it["cidx"]]) + _cost(e, it["nc"]),
        )
        st = max(clock[eng], arrival[it["cidx"]])
        merged = dict(it)
        changed = True
        while changed:
            changed = False
            for j, nx in enumerate(pend):
                if (
                    nx["gi"] == merged["gi"]
                    and nx["tj"] == merged["tj"]
                    and nx["c0"] == merged["c0"] + merged["nc"]
                    and arrival[nx["cidx"]] <= st
                    and merged["nc"] + nx["nc"] <= MERGE_CAP
                ):
                    merged["nc"] += nx["nc"]
                    merged["cidx"] = max(merged["cidx"], nx["cidx"])
                    pend.pop(j)
                    changed = True
                    break
        clock[eng] = st + _cost(eng, merged["nc"])
        work[eng].append(merged)

    # improvement search over the merged atoms: moves and swaps
    rng = np.random.default_rng(12345)
    fins = {e: replay(e, work[e]) for e in work}
    cur = max(fins.values())
    for _ in range(800):
        emax = max(fins, key=fins.get)
        if not work[emax]:
            break
        others = [e for e in work if e != emax]
        mode = rng.integers(0, 2)
        pos = int(rng.integers(0, len(work[emax])))
        tgt = others[int(rng.integers(0, len(others)))]
        w_emax = work[emax][:pos] + work[emax][pos + 1 :]
        w_tgt = work[tgt] + [work[emax][pos]]
        if mode == 1 and work[tgt]:
            pos2 = int(rng.integers(0, len(work[tgt])))
            w_tgt = work[tgt][:pos2] + work[tgt][pos2 + 1 :] + [work[emax][pos]]
            w_emax = w_emax + [work[tgt][pos2]]
        nf = dict(fins)
        nf[emax] = replay(emax, w_emax)
        nf[tgt] = replay(tgt, w_tgt)
        if max(nf.values()) < cur:
            work[emax] = w_emax
            work[tgt] = w_tgt
            fins = nf
            cur = max(nf.values())

    # final per-engine lists in arrival order, re-merged where adjacency allows
    for eng in work:
        wl = sorted(work[eng], key=key_order)
        merged_list = []
        t = start[eng]
        i = 0
        while i < len(wl):
            it = dict(wl[i])
            st = max(t, arrival[it["cidx"]])
            j = i + 1
            while (
                j < len(wl)
                and wl[j]["gi"] == it["gi"]
                and wl[j]["tj"] == it["tj"]
                and wl[j]["c0"] == it["c0"] + it["nc"]
                and arrival[wl[j]["cidx"]] <= st
            ):
                it["nc"] += wl[j]["nc"]
                it["cidx"] = max(it["cidx"], wl[j]["cidx"])
                j += 1
            t = st + _cost(eng, it["nc"])
            merged_list.append(it)
            i = j
        work[eng] = merged_list
    nd = len(work["dve"])
    na = len(work["act"])
    npo = len(work["pool"])
    S = nd + na + npo + 1

    meta = dict(
        n=n, K=K, levels=levels, counts=counts.astype(np.int64), offs=offs,
        L=L, dead=dead, groups=groups, chunks=chunks, work=work,
        nd=nd, na=na, npo=npo, S=S, G=G, ch=ch, cht=cht, chp=cht * P,
        c0sz=c0sz, combw=combw, rt=int(cp_cores[0].shape[0]),
    )
    return meta, bcols, cp_cores, ps


def _shape_key(meta):
    gkey = tuple(
        (g["rlo"], g["rhi"], g["flip"], g["clo"], g["chi"], g["bc0"], g["T"])
        for g in meta["groups"]
    )
    ckey = tuple(meta["chunks"])
    wkey = tuple(
        (eng, tuple((it["gi"], it["tj"], it["cidx"], it["c0"], it["nc"])
                    for it in meta["work"][eng]))
        for eng in ("dve", "act", "pool")
    )
    return (meta["n"], meta["L"], meta["rt"], meta["cht"], meta["c0sz"], gkey,
            ckey, wkey)


@functools.lru_cache(maxsize=8)
def _build_program(key):
    """Raw Bass program: explicit per-engine streams and semaphores."""
    n, L, rt, cht, c0sz, gkey, ckey, wkey = key
    groups = [
        dict(rlo=a, rhi=b, flip=f, clo=c, chi=d, bc0=e, T=t, ncol=d - c)
        for (a, b, f, c, d, e, t) in gkey
    ]
    chunks = list(ckey)
    work = {eng: [dict(gi=gi, tj=tj, cidx=ci, c0=c0, nc=nc_)
                  for (gi, tj, ci, c0, nc_) in wl]
            for (eng, wl) in wkey}
    nd, na, npo = len(work["dve"]), len(work["act"]), len(work["pool"])
    S = nd + na + npo + 1
    G = sum(g["T"] for g in groups)
    combw = 2 * G + 2 * cht
    NCH = len(chunks)

    tbase = {}
    b = 0
    for gi, g in enumerate(groups):
        for tj in range(g["T"]):
            tbase[(gi, tj)] = b
            b += 1

    nc = bacc.Bacc("TRN2", enable_partition_id=False)

    fp32 = mybir.dt.float32
    bf16 = mybir.dt.bfloat16
    Alu = mybir.AluOpType
    Act = mybir.ActivationFunctionType

    # hot image: [cn | cp | pred | targ | chunk0], all bf16, per-partition
    d_hot = nc.dram_tensor("hot", [(combw + c0sz) * P], bf16, kind="ExternalInput")
    d_b = nc.dram_tensor("bcols", [max(L, 1)], bf16, kind="ExternalInput")
    d_acc = nc.dram_tensor("acc", [P, S], fp32, kind="ExternalOutput")

    # bt columns: [comb (combw) | broadcast layout (L)]
    bt = nc.alloc_sbuf_tensor("bt", [P, combw + max(L, 1)], bf16)
    combf = nc.alloc_sbuf_tensor("combf", [P, max(2 * G, 1)], fp32)
    acc = nc.alloc_sbuf_tensor("acc_t", [P, S], fp32)

    max_d = max([it["nc"] for it in work["dve"]], default=1)
    max_a = max([it["nc"] for it in work["act"]], default=1)
    max_p = max([it["nc"] for it in work["pool"]], default=1)
    scr_d = [nc.alloc_sbuf_tensor(f"scr_d{i}", [P, max_d], bf16) for i in range(2)]
    scr_a = [nc.alloc_sbuf_tensor(f"scr_a{i}", [P, max_a], bf16) for i in range(2)]
    scr_p = [nc.alloc_sbuf_tensor(f"scr_p{i}", [P, max_p], fp32) for i in range(2)]

    hd = nc.alloc_sbuf_tensor("hd", [P, cht], fp32)
    hr1 = nc.alloc_sbuf_tensor("hr1", [P, cht], fp32)
    he = nc.alloc_sbuf_tensor("he", [P, cht], fp32)
    hr2 = nc.alloc_sbuf_tensor("hr2", [P, cht], fp32)
    hsq = nc.alloc_sbuf_tensor("hsq", [P, cht], fp32)
    hs1 = nc.alloc_sbuf_tensor("hs1", [P, cht], fp32)
    hs2 = nc.alloc_sbuf_tensor("hs2", [P, cht], fp32)

    s_ch = [nc.alloc_semaphore(f"s_ch{i}") for i in range(max(NCH, 1))]
    s_cv = nc.alloc_semaphore("s_cv")
    s_dve = nc.alloc_semaphore("s_dve")
    s_act = nc.alloc_semaphore("s_act")
    s_pool = nc.alloc_semaphore("s_pool")
    s_out = nc.alloc_semaphore("s_out")

    cn_col = lambda gi, tj: combf[:, tbase[(gi, tj)] : tbase[(gi, tj)] + 1]
    cp_col = lambda gi, tj: combf[:, G + tbase[(gi, tj)] : G + tbase[(gi, tj)] + 1]
    pts = bt[:, 2 * G : 2 * G + cht]
    tts = bt[:, 2 * G + cht : 2 * G + 2 * cht]

    def bcol(layout_col):
        return combw + layout_col

    def bcast_dma(sync_eng, cidx):
        lo, hi = chunks[cidx]
        src = bass.AP(tensor=d_b[:].tensor, offset=lo, ap=[[0, P], [1, hi - lo]])
        sync_eng.dma_start(out=bt[:, bcol(lo) : bcol(hi)], in_=src).then_inc(
            s_ch[cidx], 16
        )

    # --- pre-barrier DMAs: hot (comb + chunk0), then chunk 1 ---------------
    nc.sync.dma_start(
        out=bt[:, 0 : combw + c0sz],
        in_=d_hot[:].rearrange("(p t) -> p t", p=P),
    ).then_inc(s_ch[0], 16)
    if NCH > 1:
        bcast_dma(nc.sync, 1)

    def emit_stream(eng, wl, scr, mk_inst, done_sem, first_extra=None,
                    pre_waited=()):
        waited = set(pre_waited)
        last = None
        if first_extra is not None:
            first_extra()
        for k, it in enumerate(wl):
            g = groups[it["gi"]]
            lo = g["bc0"] + it["c0"]
            hi = lo + it["nc"]
            for ci, (clo, chi) in enumerate(chunks):
                if clo < hi and lo < chi and ci not in waited:
                    eng.wait_ge(s_ch[ci], 16)
                    waited.add(ci)
            if 0 not in waited:
                eng.wait_ge(s_ch[0], 16)  # comb rides chunk 0's sem
                waited.add(0)
            last = mk_inst(eng, k, it, scr[k % 2], bcol(lo), bcol(hi))
        if last is not None:
            last.then_inc(done_sem, 1)
        return last

    with nc.Block() as block:

        @block.sync
        def _(sync):
            for ci in range(2, NCH):
                bcast_dma(sync, ci)
            sync.wait_ge(s_dve, 1)
            sync.wait_ge(s_act, 1)
            sync.wait_ge(s_pool, 1)
            with nc.allow_non_contiguous_dma(reason="small accumulator tile"):
                sync.dma_start(out=d_acc[:, :], in_=acc[:, :]).then_inc(s_out, 16)
            sync.wait_ge(s_out, 16)

        @block.vector
        def _(vector):
            def mk(eng, k, it, scr, lo, hi):
                return eng.tensor_scalar(
                    out=scr[:, : it["nc"]],
                    in0=bt[:, lo:hi],
                    scalar1=cn_col(it["gi"], it["tj"]),
                    scalar2=None,
                    op0=Alu.max,
                    op1=Alu.add,
                    accum_out=acc[:, k : k + 1],
                )
            def conv():
                # upconvert the bf16 row-constant columns to f32 scalars
                vector.wait_ge(s_ch[0], 16)
                vector.tensor_scalar(
                    out=combf[:, :], in0=bt[:, 0 : 2 * G], scalar1=0.0,
                    scalar2=None, op0=Alu.add,
                ).then_inc(s_cv, 1)
                vector.drain()
            if work["dve"]:
                emit_stream(vector, work["dve"], scr_d, mk, s_dve,
                            first_extra=conv, pre_waited=(0,))
            else:
                conv()
                vector.tensor_scalar(
                    out=scr_d[0][:, 0:1], in0=bt[:, 0:1], scalar1=0.0,
                    scalar2=None, op0=Alu.add,
                ).then_inc(s_dve, 1)

        @block.scalar
        def _(act):
            def mk(eng, k, it, scr, lo, hi):
                return eng.activation(
                    out=scr[:, : it["nc"]],
                    in_=bt[:, lo:hi],
                    func=Act.Relu,
                    bias=cp_col(it["gi"], it["tj"]),
                    scale=1.0,
                    accum_out=acc[:, nd + k : nd + k + 1],
                )

            def warmup():
                # no-wait dummy so the act-table load hoists into the DMA head
                act.activation(
                    out=scr_a[0][:, 0:1], in_=scr_a[1][:, 0:1], func=Act.Relu,
                    bias=0.0, scale=1.0,
                )
                act.wait_ge(s_cv, 1)  # f32 row-constant scalars ready
            if work["act"]:
                emit_stream(act, work["act"], scr_a, mk, s_act, first_extra=warmup)
            else:
                warmup()
                act.wait_ge(s_ch[0], 16)
                act.activation(
                    out=scr_a[0][:, 0:1], in_=bt[:, 0:1], func=Act.Relu,
                    bias=0.0, scale=1.0,
                ).then_inc(s_act, 1)

        @block.gpsimd
        def _(pool):
            pool.memset(acc[:, nd + na : S], 0.0)
            pool.wait_ge(s_ch[0], 16)
            pool.tensor_tensor(out=hd[:, :], in0=pts, in1=tts, op=Alu.subtract)
            pool.tensor_scalar(
                out=hr1[:, :], in0=hd[:, :], scalar1=1.0, scalar2=0.0,
                op0=Alu.subtract, op1=Alu.max,
            )
            pool.tensor_scalar(
                out=he[:, :], in0=hd[:, :], scalar1=-1.0, scalar2=1.0,
                op0=Alu.mult, op1=Alu.subtract,
            )
            pool.tensor_scalar(
                out=hr2[:, :], in0=he[:, :], scalar1=0.0, scalar2=None,
                op0=Alu.max,
            )
            pool.tensor_tensor(out=hsq[:, :], in0=hd[:, :], in1=hd[:, :], op=Alu.mult)
            pool.tensor_tensor(out=hs1[:, :], in0=hr1[:, :], in1=hr1[:, :], op=Alu.mult)
            pool.tensor_tensor(out=hs2[:, :], in0=hr2[:, :], in1=hr2[:, :], op=Alu.mult)
            pool.tensor_tensor(out=hsq[:, :], in0=hsq[:, :], in1=hs1[:, :], op=Alu.subtract)
            pool.tensor_tensor(out=hsq[:, :], in0=hsq[:, :], in1=hs2[:, :], op=Alu.subtract)
            hub = pool.tensor_reduce(
                out=acc[0:1, S - 1 : S], in_=hsq[:, :],
                axis=mybir.AxisListType.XYZWC, op=Alu.add,
            )
            if work["pool"]:
                pool.wait_ge(s_cv, 1)  # f32 row-constant scalars ready
                waited = {0}
                last = None
                for k, it in enumerate(work["pool"]):
                    g = groups[it["gi"]]
                    lo = g["bc0"] + it["c0"]
                    hi = lo + it["nc"]
                    for ci, (clo, chi) in enumerate(chunks):
                        if clo < hi and lo < chi and ci not in waited:
                            pool.wait_ge(s_ch[ci], 16)
                            waited.add(ci)
                    scr = scr_p[k % 2]
                    pool.tensor_scalar(
                        out=scr[:, : it["nc"]], in0=bt[:, bcol(lo) : bcol(hi)],
                        scalar1=cn_col(it["gi"], it["tj"]), scalar2=None,
                        op0=Alu.max,
                    )
                    last = pool.tensor_reduce(
                        out=acc[0 : 1, nd + na + k : nd + na + k + 1],
                        in_=scr[:, : it["nc"]],
                        axis=mybir.AxisListType.XYZWC, op=Alu.add,
                    )
                last.then_inc(s_pool, 1)
            else:
                hub.then_inc(s_pool, 1)

    nc.finalize()
    return nc


def _make_inputs(meta, bcols, cp_cores, predictions, targets):
    n = meta["n"]
    cht = meta["cht"]
    chp = meta["chp"]
    L = meta["L"]
    G = meta["G"]
    c0sz = meta["c0sz"]
    bf = ml_dtypes.bfloat16
    b_all = np.ascontiguousarray(
        bcols if L > 0 else np.zeros(1, dtype=np.float32), dtype=bf
    )
    in_maps = []
    for ci in range(NCORES):
        pc = np.zeros(chp, dtype=np.float32)
        tc_ = np.zeros(chp, dtype=np.float32)
        lo = ci * meta["ch"]
        hi = min((ci + 1) * meta["ch"], n)
        if hi > lo:
            pc[: hi - lo] = predictions[lo:hi]
            tc_[: hi - lo] = targets[lo:hi]
        cp = cp_cores[ci].astype(bf)
        cols = []
        if G > 0:
            cols.append((-cp.astype(np.float32)).astype(bf).reshape(G, P).T)
            cols.append(cp.reshape(G, P).T)
        cols.append(pc.astype(bf).reshape(cht, P).T)
        cols.append(tc_.astype(bf).reshape(cht, P).T)
        cols.append(np.broadcast_to(b_all[:c0sz], (P, c0sz)))
        hot2d = np.concatenate(cols, axis=1).astype(bf)  # [128, combw + c0sz]
        in_maps.append(
            {"hot": np.ascontiguousarray(hot2d.ravel()), "bcols": b_all}
        )
    return in_maps


def _gather(meta, cp_cores, results):
    """Combine per-core accumulators into the scalar loss (float64 host math)."""
    n = meta["n"]
    groups = meta["groups"]
    work = meta["work"]
    nd = meta["nd"]
    na = meta["na"]
    S = meta["S"]

    tbase = {}
    b = 0
    for gi, g in enumerate(groups):
        for tj in range(g["T"]):
            tbase[(gi, tj)] = b
            b += 1

    num = 0.0
    hub = 0.0
    for ci in range(NCORES):
        acc = results[ci]["acc"].astype(np.float64)
        cpv = cp_cores[ci].astype(np.float64)
        for k, it in enumerate(work["dve"]):
            tb = tbase[(it["gi"], it["tj"])]
            num += acc[:, k].sum() + it["nc"] * cpv[tb * P : (tb + 1) * P].sum()
        for k, it in enumerate(work["act"]):
            num += acc[:, nd + k].sum()
        for k, it in enumerate(work["pool"]):
            tb = tbase[(it["gi"], it["tj"])]
            num += acc[0, nd + na + k] + it["nc"] * cpv[tb * P : (tb + 1) * P].sum()
        hub += acc[0, S - 1]

    huber = 0.5 * hub / n

    counts = meta["counts"].astype(np.int64)
    csum = np.cumsum(counts)
    cnt = int(np.sum(counts[1:] * csum[:-1])) if len(counts) > 1 else 0
    ranking = num / float(np.float32(cnt)) if cnt > 0 else 0.0

    return np.float32(ALPHA * huber + BETA * ranking)


def _host_fallback(predictions, targets):
    """Safety net for input distributions the device plan is not built for
    (e.g. near-continuous targets). Exact O(n^2) evaluation, row-chunked."""
    p = predictions.astype(np.float64)
    t = targets.astype(np.float64)
    n = len(p)
    d = p - t
    ad = np.abs(d)
    huber = np.mean(np.where(ad < 1.0, 0.5 * d * d, ad - 0.5))
    num = 0.0
    cnt = 0
    step = 512
    for i0 in range(0, n, step):
        i1 = min(i0 + step, n)
        pd = p[i0:i1, None] - p[None, :]
        td = t[i0:i1, None] - t[None, :]
        sign = np.where(td > 0, 1.0, -1.0)
        idx = np.arange(n)
        mask = (td != 0) & (idx[i0:i1, None] < idx[None, :])
        hinge = np.maximum(0.0, 1.0 - sign * pd)
        num += hinge[mask].sum()
        cnt += int(mask.sum())
    ranking = num / float(np.float32(cnt)) if cnt > 0 else 0.0
    return np.float32(ALPHA * huber + BETA * ranking)


def kernel(predictions: np.ndarray, targets: np.ndarray) -> np.ndarray:
    predictions = np.asarray(predictions, dtype=np.float32)
    targets = np.asarray(targets, dtype=np.float32)

    nu = len(np.unique(targets))
    if nu > 16 or nu < 2 or predictions.shape[0] < NCORES * P:
        return np.array(_host_fallback(predictions, targets), dtype=np.float32)

    meta, bcols, cp_cores, _ps = _plan(targets, predictions)
    nc = _build_program(_shape_key(meta))
    in_maps = _make_inputs(meta, bcols, cp_cores, predictions, targets)
    res = run_bass_kernel_spmd(nc, in_maps, list(range(NCORES)))
    return np.array(_gather(meta, cp_cores, res.results), dtype=np.float32)


# revision 24
# speedup vs baseline: 1.0322x; 1.0127x over previous
"""Trainium2 Bass kernel for EnhancedGradedLoss (Huber + pairwise hinge ranking).

Algorithm (see reference): loss = 0.7 * SmoothL1(p, t) + 0.3 * ranking, where
ranking averages relu(1 - sign(t_i - t_j) * (p_i - p_j)) over i<j pairs with
t_i != t_j.

Device strategy (8 NeuronCores, SPMD), v3:
  * Host sorts items by grade. Cross-grade pairs decompose via a binary split
    of the grade set: pairs(lo-set x hi-set) form one rectangular "group"
    (rows x cols), recursing into each half. A group is FLIPPED (rows = the
    lower-grade set, cols = negated upper-grade preds) when that shards into
    fewer [128 x ncol] tiles. For 4 grades this covers all 24.6M cross pairs
    in 8 row-tiles/core with ~0.5% padding waste.
  * All device data is bf16. The first DMA is a "hot" [128, combw+c0] image:
    per-row constants (-c | +c), the huber pred/targ shard, and the first
    broadcast chunk - so every engine can start as soon as one DMA lands
    (~3.3us: barrier 0.64 + HWDGE 0.63 + DGE 0.65 + transfer 0.4 + sem 0.93).
    It is issued BEFORE the block entry barrier, as is the second chunk.
    Remaining chunks stream via stride-0 broadcast DMAs (dst bytes / 360GB/s
    is the modeled wall: ~4.4us of DMA-pipe time for the 1.5MB broadcast).
  * Three engines consume the hinge tiles concurrently:
      - DVE:  tensor_scalar(max, scalar=-c, accum_out)       ~0.26 ns/col
      - ACT:  activation(Relu, bias=c, accum_out)            ~0.83 ns/col
      - Pool: tensor_scalar(max) + tensor_reduce(XYZWC)      ~2.8  ns/col
        (accum_out does not compile on GPSIMD; a full-tile reduce sums)
    using sum_j relu(B_j + c) = sum_j max(B_j, -c) + ncol * c, corrected on
    host in float64. Work items are assigned by a waterfill scheduler
    (slowest engine that still meets the modeled makespan target) honoring
    per-chunk DMA arrival times, and adjacent chunks merge into longer
    instructions once the stream runs behind the engines.
  * Huber runs entirely on Pool (prep + squares + one fused reduce).
  * ACT opens with a dummy no-wait activation so the 1.28us activation-table
    load hoists into the DMA head instead of serializing with real work.
  * One merged output DMA returns all accumulators ([128, S] f32).
"""

import functools
import sys

import ml_dtypes
import numpy as np

sys.path.insert(0, "/opt/trn_rl_repo")

import concourse.bacc as bacc
import concourse.bass as bass
from concourse import mybir
from concourse.bass_utils import run_bass_kernel_spmd

ALPHA = 0.7
BETA = 0.3
NCORES = 8
P = 128

# --- cost/latency model constants (mirrors bass_rust cost model, TRN2) -----
_HWDGE = 625.0
_DGE = 650.0
_SEM_DMA = 930.0
_T0 = 641.0                      # first HWDGE slot (after init pseudo-barrier)
_DVE_COL = 1e9 / 0.96e9 * 0.25   # 4x bf16
_DVE_FIX = 61.0
_ACT_COL = 1e9 / 1.2e9
_ACT_FIX = 372.0                 # SBUF init half + accum-read 187
_POOL_COL = 2.0 * (1e9 / 1.2e9) / 0.6   # ts-max + reduce passes
_POOL_FIX = 2.0 * 95.0 + 50.0
_HUBER_POOL = 1200.0             # memset + 9 ops + reduce
_C0 = 512                        # broadcast cols riding the hot DMA


def _cost(eng, nc_):
    if eng == "dve":
        return nc_ * _DVE_COL + _DVE_FIX
    if eng == "act":
        return nc_ * _ACT_COL + _ACT_FIX
    return nc_ * _POOL_COL + _POOL_FIX


def _plan(targets_f, predictions_f):
    """Host-side planning: sort by grade, pair-group decomposition, broadcast
    layout, DMA chunking, and 3-engine work assignment."""
    n = targets_f.shape[0]
    order = np.argsort(targets_f, kind="stable")
    ts = targets_f[order]
    ps = predictions_f[order].astype(np.float32)

    levels, counts = np.unique(ts, return_counts=True)
    K = len(levels)
    offs = np.concatenate([[0], np.cumsum(counts)]).astype(np.int64)

    pmax = float(np.max(np.abs(ps))) if n else 0.0
    dead = -float(np.float32(np.ceil(pmax) + 2.0))

    # --- pair groups via binary grade split, with per-group flip choice ----
    def tiles_of(m):
        q = -(-m // NCORES)
        return -(-q // P)

    groups = []

    def rec(a, b):
        if b - a < 2:
            return
        mid = (a + b) // 2
        m_un = int(offs[b] - offs[mid])
        m_fl = int(offs[mid] - offs[a])
        ncol_un = int(offs[mid] - offs[a])
        ncol_fl = int(offs[b] - offs[mid])
        if m_un and ncol_un:
            if tiles_of(m_fl) * ncol_fl < tiles_of(m_un) * ncol_un:
                groups.append(
                    dict(rlo=int(offs[a]), rhi=int(offs[mid]), flip=True,
                         clo=int(offs[mid]), chi=int(offs[b]))
                )
            else:
                groups.append(
                    dict(rlo=int(offs[mid]), rhi=int(offs[b]), flip=False,
                         clo=int(offs[a]), chi=int(offs[mid]))
                )
        rec(a, mid)
        rec(mid, b)

    rec(0, K)

    # --- broadcast layout --------------------------------------------------
    placed = []
    cursor = 0
    for g in sorted(groups, key=lambda g: (g["flip"], -(g["chi"] - g["clo"]))):
        sgn = -1 if g["flip"] else 1
        hit = None
        for (s2, lo2, hi2, st2) in placed:
            if s2 == sgn and lo2 <= g["clo"] and g["chi"] <= hi2:
                hit = st2 + (g["clo"] - lo2)
                break
        if hit is None:
            hit = cursor
            placed.append((sgn, g["clo"], g["chi"], cursor))
            cursor += g["chi"] - g["clo"]
        g["bc0"] = int(hit)
    L = cursor

    bcols = np.zeros(max(L, 1), dtype=np.float32)
    for (sgn, lo, hi, st) in placed:
        bcols[st : st + (hi - lo)] = sgn * ps[lo:hi]

    for g in groups:
        m = g["rhi"] - g["rlo"]
        g["m"] = m
        g["q"] = -(-m // NCORES)
        g["T"] = -(-g["q"] // P)
        g["ncol"] = g["chi"] - g["clo"]

    G = sum(g["T"] for g in groups)
    ch = -(-n // NCORES)
    cht = -(-ch // P)
    combw = 2 * G + 2 * cht

    # per-core +c row constants, bf16-rounded (device and host use the same)
    cp_cores = []
    for ci in range(NCORES):
        parts = []
        for g in groups:
            r0 = g["rlo"] + ci * g["q"]
            r1 = min(g["rlo"] + min((ci + 1) * g["q"], g["m"]), g["rhi"])
            r0 = min(r0, r1)
            pv = ps[r0:r1]
            c = (np.float32(1.0) + pv) if g["flip"] else (np.float32(1.0) - pv)
            padded = np.full(g["T"] * P, dead, dtype=np.float32)
            padded[: len(c)] = c.astype(np.float32)
            parts.append(padded.astype(ml_dtypes.bfloat16))
        cp_cores.append(
            np.concatenate(parts) if parts else np.zeros(0, dtype=ml_dtypes.bfloat16)
        )

    # --- DMA chunk plan ----------------------------------------------------
    # chunk 0 (size ~_C0) rides the hot DMA; early chunks are smaller so the
    # compute stream ramps quickly, later ones bigger; cut at group
    # boundaries; small final chunk to cut tail latency.
    c0sz = min(_C0, L)
    cuts = {0, c0sz, L}
    for g in groups:
        cuts.add(g["bc0"])
        cuts.add(g["bc0"] + g["ncol"])
    cuts = sorted(c for c in cuts if 0 <= c <= L)
    chunks = []
    for lo, hi in zip(cuts[:-1], cuts[1:]):
        span = hi - lo
        if span <= 0:
            continue
        tgt = 800 if lo < 2000 else 1100
        k = max(1, -(-span // tgt))
        bnds = [lo + span * j // k for j in range(k + 1)]
        for j in range(k):
            chunks.append((bnds[j], bnds[j + 1]))
    # split the final chunk so the last-arriving piece is small
    if chunks and chunks[-1][1] - chunks[-1][0] > 700:
        lo, hi = chunks.pop()
        chunks.append((lo, hi - 384))
        chunks.append((hi - 384, hi))
    NCH = len(chunks)

    # --- arrival model (calibrated against TimelineSim traces) -------------
    # DMA order: hot (comb + chunk0), chunk1.., first two pre-block.
    arrival = [0.0] * NCH
    prev_h = _T0
    prev_d = 0.0
    hot_bytes = (combw + c0sz) * 2 * P
    plan_order = [-1] + list(range(1, NCH))  # -1 = hot (includes chunk 0)
    for oi, idx in enumerate(plan_order):
        h_end = prev_h + 650.0 + (50.0 if oi == 2 else 0.0)
        prev_h = h_end
        if idx == -1:
            nbytes = hot_bytes
            elem = (combw + c0sz) * 2
        else:
            lo, hi = chunks[idx]
            nbytes = (hi - lo) * 2 * P
            elem = (hi - lo) * 2
        mult = 2.0 if elem < 512 else 1.0
        tr = max(nbytes * mult / 360.0, 7.0)
        d_end = max(h_end + _DGE, prev_d) + tr
        prev_d = d_end
        sem_t = d_end + _SEM_DMA
        if idx == -1:
            arrival[0] = sem_t
            comb_arrival = sem_t
        else:
            arrival[idx] = sem_t

    # --- work items --------------------------------------------------------
    items = []
    for gi, g in enumerate(groups):
        glo, ghi = g["bc0"], g["bc0"] + g["ncol"]
        for cidx, (lo, hi) in enumerate(chunks):
            s, e = max(lo, glo), min(hi, ghi)
            if s >= e:
                continue
            for tj in range(g["T"]):
                items.append(dict(gi=gi, tj=tj, cidx=cidx, c0=s - glo, nc=e - s))
    items.sort(key=lambda it: (arrival[it["cidx"]], it["gi"], it["tj"]))

    start = {
        "dve": comb_arrival + 116.0,   # conv + drain
        "act": comb_arrival + 142.0,   # s_cv hop
        "pool": comb_arrival + 1095.0, # huber block
    }

    key_order = lambda it: (arrival[it["cidx"]], it["gi"], it["tj"], it["c0"])

    def replay(eng, wl):
        """Merge-aware finish time: adjacent same-tile items merge when the
        later chunk has arrived by the time the run starts."""
        t = start[eng]
        i = 0
        wl = sorted(wl, key=key_order)
        while i < len(wl):
            it = wl[i]
            st = max(t, arrival[it["cidx"]])
            nc_ = it["nc"]
            j = i + 1
            while (
                j < len(wl)
                and wl[j]["gi"] == it["gi"]
                and wl[j]["tj"] == it["tj"]
                and wl[j]["c0"] == it["c0"] + nc_
                and arrival[wl[j]["cidx"]] <= st
            ):
                nc_ += wl[j]["nc"]
                j += 1
            t = st + _cost(eng, nc_)
            i = j
        return t

    # initial: earliest-finish-time at chunk granularity with inline merging
    MERGE_CAP = 2200
    clock = dict(start)
    pend = [dict(it) for it in items]
    work = {"dve": [], "act": [], "pool": []}
    while pend:
        it = pend.pop(0)
        eng = min(
            ("dve", "act", "pool"),
            key=lambda e: max(clock[e], arrival[it["cidx"]]) + _cost(e, it["nc"]),
        )
        st = max(clock[eng], arrival[it["cidx"]])
        merged = dict(it)
        changed = True
        while changed:
            changed = False
            for j, nx in enumerate(pend):
                if (
                    nx["gi"] == merged["gi"]
                    and nx["tj"] == merged["tj"]
                    and nx["c0"] == merged["c0"] + merged["nc"]
                    and arrival[nx["cidx"]] <= st
                    and merged["nc"] + nx["nc"] <= MERGE_CAP
                ):
                    merged["nc"] += nx["nc"]
                    merged["cidx"] = max(merged["cidx"], nx["cidx"])
                    pend.pop(j)
                    changed = True
                    break
        clock[eng] = st + _cost(eng, merged["nc"])
        work[eng].append(merged)

    # improvement search over the merged atoms: moves, swaps, splits
    def split_atom(it):
        """Split a merged atom at an interior chunk boundary, if any."""
        g = groups[it["gi"]]
        lo = g["bc0"] + it["c0"]
        hi = lo + it["nc"]
        for (clo, chi) in chunks:
            if lo < chi < hi:
                ci_a = next(i for i, (a, b) in enumerate(chunks)
                            if a < chi <= b)
                ci_b = it["cidx"]
                a = dict(it, nc=chi - lo, cidx=ci_a)
                b = dict(it, c0=it["c0"] + (chi - lo), nc=hi - chi, cidx=ci_b)
                return a, b
        return None

    best_work = None
    best_mk = float("inf")
    for seed in (12345, 777, 31337):
        rng = np.random.default_rng(seed)
        wtrial = {e: [dict(it) for it in work[e]] for e in work}
        fins = {e: replay(e, wtrial[e]) for e in wtrial}
        cur = max(fins.values())
        for _ in range(2500):
            emax = max(fins, key=fins.get)
            if not wtrial[emax]:
                break
            others = [e for e in wtrial if e != emax]
            mode = int(rng.integers(0, 3))
            pos = int(rng.integers(0, len(wtrial[emax])))
            tgt = others[int(rng.integers(0, len(others)))]
            if mode == 2:
                sp = split_atom(wtrial[emax][pos])
                if sp is None:
                    continue
                a, b = sp
                w_emax = wtrial[emax][:pos] + wtrial[emax][pos + 1 :] + [a]
                w_tgt = wtrial[tgt] + [b]
            else:
                w_emax = wtrial[emax][:pos] + wtrial[emax][pos + 1 :]
                w_tgt = wtrial[tgt] + [wtrial[emax][pos]]
                if mode == 1 and wtrial[tgt]:
                    pos2 = int(rng.integers(0, len(wtrial[tgt])))
                    w_tgt = (wtrial[tgt][:pos2] + wtrial[tgt][pos2 + 1 :]
                             + [wtrial[emax][pos]])
                    w_emax = w_emax + [wtrial[tgt][pos2]]
            nf = dict(fins)
            nf[emax] = replay(emax, w_emax)
            nf[tgt] = replay(tgt, w_tgt)
            if max(nf.values()) < cur:
                wtrial[emax] = w_emax
                wtrial[tgt] = w_tgt
                fins = nf
                cur = max(nf.values())
        if cur < best_mk:
            best_mk = cur
            best_work = wtrial
    work = best_work

    # final per-engine lists in arrival order, re-merged where adjacency allows
    for eng in work:
        wl = sorted(work[eng], key=key_order)
        merged_list = []
        t = start[eng]
        i = 0
        while i < len(wl):
            it = dict(wl[i])
            st = max(t, arrival[it["cidx"]])
            j = i + 1
            while (
                j < len(wl)
                and wl[j]["gi"] == it["gi"]
                and wl[j]["tj"] == it["tj"]
                and wl[j]["c0"] == it["c0"] + it["nc"]
                and arrival[wl[j]["cidx"]] <= st
            ):
                it["nc"] += wl[j]["nc"]
                it["cidx"] = max(it["cidx"], wl[j]["cidx"])
                j += 1
            t = st + _cost(eng, it["nc"])
            merged_list.append(it)
            i = j
        work[eng] = merged_list
    nd = len(work["dve"])
    na = len(work["act"])
    npo = len(work["pool"])
    S = nd + na + npo + 1

    meta = dict(
        n=n, K=K, levels=levels, counts=counts.astype(np.int64), offs=offs,
        L=L, dead=dead, groups=groups, chunks=chunks, work=work,
        nd=nd, na=na, npo=npo, S=S, G=G, ch=ch, cht=cht, chp=cht * P,
        c0sz=c0sz, combw=combw, rt=int(cp_cores[0].shape[0]),
    )
    return meta, bcols, cp_cores, ps


def _shape_key(meta):
    gkey = tuple(
        (g["rlo"], g["rhi"], g["flip"], g["clo"], g["chi"], g["bc0"], g["T"])
        for g in meta["groups"]
    )
    ckey = tuple(meta["chunks"])
    wkey = tuple(
        (eng, tuple((it["gi"], it["tj"], it["cidx"], it["c0"], it["nc"])
                    for it in meta["work"][eng]))
        for eng in ("dve", "act", "pool")
    )
    return (meta["n"], meta["L"], meta["rt"], meta["cht"], meta["c0sz"], gkey,
            ckey, wkey)


@functools.lru_cache(maxsize=8)
def _build_program(key):
    """Raw Bass program: explicit per-engine streams and semaphores."""
    n, L, rt, cht, c0sz, gkey, ckey, wkey = key
    groups = [
        dict(rlo=a, rhi=b, flip=f, clo=c, chi=d, bc0=e, T=t, ncol=d - c)
        for (a, b, f, c, d, e, t) in gkey
    ]
    chunks = list(ckey)
    work = {eng: [dict(gi=gi, tj=tj, cidx=ci, c0=c0, nc=nc_)
                  for (gi, tj, ci, c0, nc_) in wl]
            for (eng, wl) in wkey}
    nd, na, npo = len(work["dve"]), len(work["act"]), len(work["pool"])
    S = nd + na + npo + 1
    G = sum(g["T"] for g in groups)
    combw = 2 * G + 2 * cht
    NCH = len(chunks)

    tbase = {}
    b = 0
    for gi, g in enumerate(groups):
        for tj in range(g["T"]):
            tbase[(gi, tj)] = b
            b += 1

    nc = bacc.Bacc("TRN2", enable_partition_id=False)

    fp32 = mybir.dt.float32
    bf16 = mybir.dt.bfloat16
    Alu = mybir.AluOpType
    Act = mybir.ActivationFunctionType

    # hot image: [cn | cp | pred | targ | chunk0], all bf16, per-partition
    d_hot = nc.dram_tensor("hot", [(combw + c0sz) * P], bf16, kind="ExternalInput")
    d_b = nc.dram_tensor("bcols", [max(L, 1)], bf16, kind="ExternalInput")
    d_acc = nc.dram_tensor("acc", [P, S], fp32, kind="ExternalOutput")

    # bt columns: [comb (combw) | broadcast layout (L)]
    bt = nc.alloc_sbuf_tensor("bt", [P, combw + max(L, 1)], bf16)
    combf = nc.alloc_sbuf_tensor("combf", [P, max(2 * G, 1)], fp32)
    acc = nc.alloc_sbuf_tensor("acc_t", [P, S], fp32)

    max_d = max([it["nc"] for it in work["dve"]], default=1)
    max_a = max([it["nc"] for it in work["act"]], default=1)
    max_p = max([it["nc"] for it in work["pool"]], default=1)
    scr_d = [nc.alloc_sbuf_tensor(f"scr_d{i}", [P, max_d], bf16) for i in range(2)]
    scr_a = [nc.alloc_sbuf_tensor(f"scr_a{i}", [P, max_a], bf16) for i in range(2)]
    scr_p = [nc.alloc_sbuf_tensor(f"scr_p{i}", [P, max_p], fp32) for i in range(2)]

    hd = nc.alloc_sbuf_tensor("hd", [P, cht], fp32)
    hr1 = nc.alloc_sbuf_tensor("hr1", [P, cht], fp32)
    he = nc.alloc_sbuf_tensor("he", [P, cht], fp32)
    hr2 = nc.alloc_sbuf_tensor("hr2", [P, cht], fp32)
    hsq = nc.alloc_sbuf_tensor("hsq", [P, cht], fp32)
    hs1 = nc.alloc_sbuf_tensor("hs1", [P, cht], fp32)
    hs2 = nc.alloc_sbuf_tensor("hs2", [P, cht], fp32)

    s_ch = [nc.alloc_semaphore(f"s_ch{i}") for i in range(max(NCH, 1))]
    s_cv = nc.alloc_semaphore("s_cv")
    s_dve = nc.alloc_semaphore("s_dve")
    s_act = nc.alloc_semaphore("s_act")
    s_pool = nc.alloc_semaphore("s_pool")
    s_out = nc.alloc_semaphore("s_out")

    cn_col = lambda gi, tj: combf[:, tbase[(gi, tj)] : tbase[(gi, tj)] + 1]
    cp_col = lambda gi, tj: combf[:, G + tbase[(gi, tj)] : G + tbase[(gi, tj)] + 1]
    pts = bt[:, 2 * G : 2 * G + cht]
    tts = bt[:, 2 * G + cht : 2 * G + 2 * cht]

    def bcol(layout_col):
        return combw + layout_col

    def bcast_dma(sync_eng, cidx):
        lo, hi = chunks[cidx]
        src = bass.AP(tensor=d_b[:].tensor, offset=lo, ap=[[0, P], [1, hi - lo]])
        sync_eng.dma_start(out=bt[:, bcol(lo) : bcol(hi)], in_=src).then_inc(
            s_ch[cidx], 16
        )

    # --- pre-barrier DMAs: hot (comb + chunk0), then chunk 1 ---------------
    nc.sync.dma_start(
        out=bt[:, 0 : combw + c0sz],
        in_=d_hot[:].rearrange("(p t) -> p t", p=P),
    ).then_inc(s_ch[0], 16)
    if NCH > 1:
        bcast_dma(nc.sync, 1)

    def emit_stream(eng, wl, scr, mk_inst, done_sem, first_extra=None,
                    pre_waited=()):
        waited = set(pre_waited)
        last = None
        if first_extra is not None:
            first_extra()
        for k, it in enumerate(wl):
            g = groups[it["gi"]]
            lo = g["bc0"] + it["c0"]
            hi = lo + it["nc"]
            for ci, (clo, chi) in enumerate(chunks):
                if clo < hi and lo < chi and ci not in waited:
                    eng.wait_ge(s_ch[ci], 16)
                    waited.add(ci)
            if 0 not in waited:
                eng.wait_ge(s_ch[0], 16)  # comb rides chunk 0's sem
                waited.add(0)
            last = mk_inst(eng, k, it, scr[k % 2], bcol(lo), bcol(hi))
        if last is not None:
            last.then_inc(done_sem, 1)
        return last

    with nc.Block() as block:

        @block.sync
        def _(sync):
            for ci in range(2, NCH):
                bcast_dma(sync, ci)
            sync.wait_ge(s_dve, 1)
            sync.wait_ge(s_act, 1)
            sync.wait_ge(s_pool, 1)
            with nc.allow_non_contiguous_dma(reason="small accumulator tile"):
                sync.dma_start(out=d_acc[:, :], in_=acc[:, :]).then_inc(s_out, 16)
            sync.wait_ge(s_out, 16)

        @block.vector
        def _(vector):
            def mk(eng, k, it, scr, lo, hi):
                return eng.tensor_scalar(
                    out=scr[:, : it["nc"]],
                    in0=bt[:, lo:hi],
                    scalar1=cn_col(it["gi"], it["tj"]),
                    scalar2=None,
                    op0=Alu.max,
                    op1=Alu.add,
                    accum_out=acc[:, k : k + 1],
                )
            def conv():
                # upconvert the bf16 row-constant columns to f32 scalars
                vector.wait_ge(s_ch[0], 16)
                vector.tensor_scalar(
                    out=combf[:, :], in0=bt[:, 0 : 2 * G], scalar1=0.0,
                    scalar2=None, op0=Alu.add,
                ).then_inc(s_cv, 1)
                vector.drain()
            if work["dve"]:
                emit_stream(vector, work["dve"], scr_d, mk, s_dve,
                            first_extra=conv, pre_waited=(0,))
            else:
                conv()
                vector.tensor_scalar(
                    out=scr_d[0][:, 0:1], in0=bt[:, 0:1], scalar1=0.0,
                    scalar2=None, op0=Alu.add,
                ).then_inc(s_dve, 1)

        @block.scalar
        def _(act):
            def mk(eng, k, it, scr, lo, hi):
                return eng.activation(
                    out=scr[:, : it["nc"]],
                    in_=bt[:, lo:hi],
                    func=Act.Relu,
                    bias=cp_col(it["gi"], it["tj"]),
                    scale=1.0,
                    accum_out=acc[:, nd + k : nd + k + 1],
                )

            def warmup():
                # no-wait dummy so the act-table load hoists into the DMA head
                act.activation(
                    out=scr_a[0][:, 0:1], in_=scr_a[1][:, 0:1], func=Act.Relu,
                    bias=0.0, scale=1.0,
                )
                act.wait_ge(s_cv, 1)  # f32 row-constant scalars ready
            if work["act"]:
                emit_stream(act, work["act"], scr_a, mk, s_act, first_extra=warmup)
            else:
                warmup()
                act.wait_ge(s_ch[0], 16)
                act.activation(
                    out=scr_a[0][:, 0:1], in_=bt[:, 0:1], func=Act.Relu,
                    bias=0.0, scale=1.0,
                ).then_inc(s_act, 1)

        @block.gpsimd
        def _(pool):
            pool.memset(acc[:, nd + na : S], 0.0)
            pool.wait_ge(s_ch[0], 16)
            pool.tensor_tensor(out=hd[:, :], in0=pts, in1=tts, op=Alu.subtract)
            pool.tensor_scalar(
                out=hr1[:, :], in0=hd[:, :], scalar1=1.0, scalar2=0.0,
                op0=Alu.subtract, op1=Alu.max,
            )
            pool.tensor_scalar(
                out=he[:, :], in0=hd[:, :], scalar1=-1.0, scalar2=1.0,
                op0=Alu.mult, op1=Alu.subtract,
            )
            pool.tensor_scalar(
                out=hr2[:, :], in0=he[:, :], scalar1=0.0, scalar2=None,
                op0=Alu.max,
            )
            pool.tensor_tensor(out=hsq[:, :], in0=hd[:, :], in1=hd[:, :], op=Alu.mult)
            pool.tensor_tensor(out=hs1[:, :], in0=hr1[:, :], in1=hr1[:, :], op=Alu.mult)
            pool.tensor_tensor(out=hs2[:, :], in0=hr2[:, :], in1=hr2[:, :], op=Alu.mult)
            pool.tensor_tensor(out=hsq[:, :], in0=hsq[:, :], in1=hs1[:, :], op=Alu.subtract)
            pool.tensor_tensor(out=hsq[:, :], in0=hsq[:, :], in1=hs2[:, :], op=Alu.subtract)
            hub = pool.tensor_reduce(
                out=acc[0:1, S - 1 : S], in_=hsq[:, :],
                axis=mybir.AxisListType.XYZWC, op=Alu.add,
            )
            if work["pool"]:
                pool.wait_ge(s_cv, 1)  # f32 row-constant scalars ready
                waited = {0}
                last = None
                for k, it in enumerate(work["pool"]):
                    g = groups[it["gi"]]
                    lo = g["bc0"] + it["c0"]
                    hi = lo + it["nc"]
                    for ci, (clo, chi) in enumerate(chunks):
                        if clo < hi and lo < chi and ci not in waited:
                            pool.wait_ge(s_ch[ci], 16)
                            waited.add(ci)
                    scr = scr_p[k % 2]
                    pool.tensor_scalar(
                        out=scr[:, : it["nc"]], in0=bt[:, bcol(lo) : bcol(hi)],
                        scalar1=cn_col(it["gi"], it["tj"]), scalar2=None,
                        op0=Alu.max,
                    )
                    last = pool.tensor_reduce(
                        out=acc[0 : 1, nd + na + k : nd + na + k + 1],
                        in_=scr[:, : it["nc"]],
                        axis=mybir.AxisListType.XYZWC, op=Alu.add,
                    )
                last.then_inc(s_pool, 1)
            else:
                hub.then_inc(s_pool, 1)

    nc.finalize()
    return nc


def _make_inputs(meta, bcols, cp_cores, predictions, targets):
    n = meta["n"]
    cht = meta["cht"]
    chp = meta["chp"]
    L = meta["L"]
    G = meta["G"]
    c0sz = meta["c0sz"]
    bf = ml_dtypes.bfloat16
    b_all = np.ascontiguousarray(
        bcols if L > 0 else np.zeros(1, dtype=np.float32), dtype=bf
    )
    in_maps = []
    for ci in range(NCORES):
        pc = np.zeros(chp, dtype=np.float32)
        tc_ = np.zeros(chp, dtype=np.float32)
        lo = ci * meta["ch"]
        hi = min((ci + 1) * meta["ch"], n)
        if hi > lo:
            pc[: hi - lo] = predictions[lo:hi]
            tc_[: hi - lo] = targets[lo:hi]
        cp = cp_cores[ci].astype(bf)
        cols = []
        if G > 0:
            cols.append((-cp.astype(np.float32)).astype(bf).reshape(G, P).T)
            cols.append(cp.reshape(G, P).T)
        cols.append(pc.astype(bf).reshape(cht, P).T)
        cols.append(tc_.astype(bf).reshape(cht, P).T)
        cols.append(np.broadcast_to(b_all[:c0sz], (P, c0sz)))
        hot2d = np.concatenate(cols, axis=1).astype(bf)  # [128, combw + c0sz]
        in_maps.append(
            {"hot": np.ascontiguousarray(hot2d.ravel()), "bcols": b_all}
        )
    return in_maps


def _gather(meta, cp_cores, results):
    """Combine per-core accumulators into the scalar loss (float64 host math)."""
    n = meta["n"]
    groups = meta["groups"]
    work = meta["work"]
    nd = meta["nd"]
    na = meta["na"]
    S = meta["S"]

    tbase = {}
    b = 0
    for gi, g in enumerate(groups):
        for tj in range(g["T"]):
            tbase[(gi, tj)] = b
            b += 1

    num = 0.0
    hub = 0.0
    for ci in range(NCORES):
        acc = results[ci]["acc"].astype(np.float64)
        cpv = cp_cores[ci].astype(np.float64)
        for k, it in enumerate(work["dve"]):
            tb = tbase[(it["gi"], it["tj"])]
            num += acc[:, k].sum() + it["nc"] * cpv[tb * P : (tb + 1) * P].sum()
        for k, it in enumerate(work["act"]):
            num += acc[:, nd + k].sum()
        for k, it in enumerate(work["pool"]):
            tb = tbase[(it["gi"], it["tj"])]
            num += acc[0, nd + na + k] + it["nc"] * cpv[tb * P : (tb + 1) * P].sum()
        hub += acc[0, S - 1]

    huber = 0.5 * hub / n

    counts = meta["counts"].astype(np.int64)
    csum = np.cumsum(counts)
    cnt = int(np.sum(counts[1:] * csum[:-1])) if len(counts) > 1 else 0
    ranking = num / float(np.float32(cnt)) if cnt > 0 else 0.0

    return np.float32(ALPHA * huber + BETA * ranking)


def _host_fallback(predictions, targets):
    """Safety net for input distributions the device plan is not built for
    (e.g. near-continuous targets). Exact O(n^2) evaluation, row-chunked."""
    p = predictions.astype(np.float64)
    t = targets.astype(np.float64)
    n = len(p)
    d = p - t
    ad = np.abs(d)
    huber = np.mean(np.where(ad < 1.0, 0.5 * d * d, ad - 0.5))
    num = 0.0
    cnt = 0
    step = 512
    for i0 in range(0, n, step):
        i1 = min(i0 + step, n)
        pd = p[i0:i1, None] - p[None, :]
        td = t[i0:i1, None] - t[None, :]
        sign = np.where(td > 0, 1.0, -1.0)
        idx = np.arange(n)
        mask = (td != 0) & (idx[i0:i1, None] < idx[None, :])
        hinge = np.maximum(0.0, 1.0 - sign * pd)
        num += hinge[mask].sum()
        cnt += int(mask.sum())
    ranking = num / float(np.float32(cnt)) if cnt > 0 else 0.0
    return np.float32(ALPHA * huber + BETA * ranking)


def kernel(predictions: np.ndarray, targets: np.ndarray) -> np.ndarray:
    predictions = np.asarray(predictions, dtype=np.float32)
    targets = np.asarray(targets, dtype=np.float32)

    nu = len(np.unique(targets))
    if nu > 16 or nu < 2 or predictions.shape[0] < NCORES * P:
        return np.array(_host_fallback(predictions, targets), dtype=np.float32)

    meta, bcols, cp_cores, _ps = _plan(targets, predictions)
    nc = _build_program(_shape_key(meta))
    in_maps = _make_inputs(meta, bcols, cp_cores, predictions, targets)
    res = run_bass_kernel_spmd(nc, in_maps, list(range(NCORES)))
    return np.array(_gather(meta, cp_cores, res.results), dtype=np.float32)
